# revision 32
# baseline (speedup 1.0000x reference)
"""MiniBindingAttention Trainium2 kernel.

Reference computation (per batch b, head h, T=2048, HD=64):
    Q = x_h * sign(bv_q); K = x_h * sign(bv_k); V = x_h * sign(bv_v)
    scores = Q @ K.T / sqrt(HD)
    attn   = causal ? sigmoid(4 * scores) : 0
    out    = attn @ V

Key algebra / layout:
  - sigmoid(4*scale*QK) = sigmoid((x_q . x_k) * 0.5 * sq*sk) with sq*sk in
    {+-1} -> fold 0.5*sign(bv_q)*sign(bv_k) into one scaled copy of x.
  - sign(bv_v) is folded into V on the host (xNv), so no per-channel fixup
    is needed on-chip for the output path.
  - scores are computed TRANSPOSED ([k, q] layout) so the second matmul
    (contraction over k) needs no on-chip transposes; the host supplies
    x transposed+duplicated along partitions (two k-tiles' score matmuls
    run concurrently in disjoint PE row-groups) plus a swizzled natural
    copy for the V side.
  - all on-chip operands are bf16 (halves DMA + SBUF; matmul is 1 cyc/row
    for bf16 at any free size, so diagonal trims are free).
  - score chunks are packed into large [128, 1536] f32 PSUM tiles (3 banks)
    so ONE sigmoid activation covers 1536 columns: the scalar engine is the
    critical resource and each ACTIVATE carries ~320ns of fixed cost, so 54
    big activations beat the 112 small ones of the per-wave formulation.
    (Trimming causally-dead columns out of the score matmuls as well dies
    with an exec-unit crash on real HW — sim-clean; left disabled.)
  - the second matmul is column-packed: even/odd k-tiles accumulate
    concurrently in disjoint PE column-groups (partitions 0:64 / 64:128 of
    two separate PSUM banks — separate so the two streams do not contend on
    one bank's write port); the fold is a DVE copy of the odd half to SBUF
    plus a DVE scalar_tensor_tensor add (DMA cannot read PSUM, and plain
    tensor_tensor with a PSUM operand crashes the DVE).
  - causal masking inside diagonal 128x128 blocks: DVE multiplies the
    post-sigmoid block by a 0/1 staircase constant (GpSimd is slower at
    this and cannot touch PSUM at all).

Sharding: B*H = 32 (batch, head) pairs, 4 per core across 8 cores.
"""

import numpy as np
import ml_dtypes

import concourse.tile as tile
from concourse import bacc, mybir
from concourse.bass_utils import run_bass_kernel_spmd

N_CORES = 8
B, T, D, H, HD = 2, 2048, 1024, 16, 64
PAIRS = (B * H) // N_CORES  # 4 (b,h) pairs per core
KT = T // 128               # 16 k-tiles of 128 rows
QB = T // 512               # 4 q-blocks of 512 cols
SCW = 1536                  # score-tile width (3 PSUM banks of f32)
F32 = mybir.dt.float32
BF16 = mybir.dt.bfloat16
SIG = mybir.ActivationFunctionType.Sigmoid


def _chunks():
    """Flat causal chunk list: (pair, q-block, k-tile, off, off_sc).

    `off` is the first causally-live column of the chunk's 512-col q-window;
    `off_sc` (0 or 256) is the column where the score matmul actually starts
    — trimming only at 256 granularity keeps every score chunk a whole
    multiple of half a PSUM bank, so packed chunks never cross a bank
    boundary (which matmul outputs cannot do).
    """
    out = []
    for p in range(PAIRS):
        for j in range(QB):
            for i in range(4 * j + 4):
                off = 128 * (i - 4 * j) if i > 4 * j else 0
                off_sc = 0  # trims crash on HW (lowering edge?); disabled
                out.append((p, j, i, off, off_sc))
    return out


def _groups():
    """Pack chunks contiguously into score tiles of <= SCW live columns.

    Returns a list of groups; each group is (chunk_list, width) where
    chunk_list holds (chunk, cursor) pairs giving each chunk's column
    position inside the tile.
    """
    raw, cur_list = [], []
    for n, ch in enumerate(_chunks()):
        if len(cur_list) * 512 + 512 > SCW or n == 1:
            raw.append(cur_list)
            cur_list = []
        cur_list.append(ch)
    raw.append(cur_list)
    # within each group, place the chunk with the largest causally-dead
    # prefix FIRST so the activation can skip it (in_=sc[:, off0:gw]);
    # mm2 emission keeps the original k-order, so accumulation start/stop
    # flags are unaffected by the placement order
    groups = []
    for chs in raw:
        chs = sorted(chs, key=lambda c: -c[3])
        placed = [(ch, 512 * s) for s, ch in enumerate(chs)]
        groups.append((placed, 512 * len(chs), chs[0][3]))
    return groups


def build():
    nc = bacc.Bacc("TRN2", target_bir_lowering=False)
    # xT duplicated along partitions: [0:64]=x^T, [64:128]=x^T (row-group pack)
    xT_d = nc.dram_tensor("xT", [PAIRS, 128, T], BF16, kind="ExternalInput")
    # wxT = xT * (0.5*sign(bv_q)*sign(bv_k)) precomputed on host
    wxT_d = nc.dram_tensor("wxT", [PAIRS, 128, T], BF16, kind="ExternalInput")
    # xN pre-swizzled on host, sign(bv_v) folded in:
    #   xN[p, pp, 64*k+d] = x[128*k+pp, d] * sv[d]
    xN_d = nc.dram_tensor("xN", [PAIRS, 128, KT * HD], BF16, kind="ExternalInput")
    # stair01[p, n] = 0.0 if n < p else 1.0 (keep mask for diagonal blocks)
    msk_d = nc.dram_tensor("msk", [128, 128], BF16, kind="ExternalInput")
    out_d = nc.dram_tensor("outT", [PAIRS, HD, T], F32, kind="ExternalOutput")

    with tile.TileContext(nc) as tc:
        with (
            tc.tile_pool(name="consts", bufs=1) as consts,
            tc.tile_pool(name="xpool", bufs=4) as xpool,
            tc.tile_pool(name="attnp", bufs=4) as attnp,
            tc.tile_pool(name="outp", bufs=3) as outp,
            tc.tile_pool(name="psum_s", bufs=2, space="PSUM") as psum_s,
            tc.tile_pool(name="psum_o", bufs=1, space="PSUM") as psum_o,
        ):
            stair = consts.tile([128, 128], BF16)
            nc.sync.dma_start(out=stair, in_=msk_d[:])

            state = {}

            def load_pair(p):
                xT = xpool.tile([128, T], BF16, tag="xT")
                wxT = xpool.tile([128, T], BF16, tag="wxT")
                # chunked loads give the scheduler finer-grained
                # dependencies; pair 0 gets a small first chunk so the
                # first score matmul starts as early as possible
                bounds = [0, 512, 1024, 2048] if p == 0 else [0, 1024, 2048]
                for lo, hi in zip(bounds, bounds[1:]):
                    cs = slice(lo, hi)
                    nc.sync.dma_start(out=wxT[:, cs], in_=wxT_d[p, :, cs])
                    nc.sync.dma_start(out=xT[:, cs], in_=xT_d[p, :, cs])
                xN = xpool.tile([128, KT * HD], BF16, tag="xN")
                nc.sync.dma_start(out=xN, in_=xN_d[p])
                state[p] = (xT, xN, wxT)

            oaccs = {}      # (p, j) -> [128, 512] psum accumulator
            pending = None  # deferred mm2 work: (chunk_list, att)

            def emit_mm2(chunk_list, att):
                for (p, j, i, off, off_sc), cur in sorted(chunk_list, key=lambda c: c[0][:3]):
                    _, xN, _ = state[p]
                    if i == 0:
                        oaccA = psum_o.tile([128, 512], F32, name="oaccA", tag="oaccA")
                        oaccB = psum_o.tile([128, 512], F32, name="oaccB", tag="oaccB")
                        oaccs[(p, j)] = (oaccA, oaccB)
                        if j == 0:
                            # cols 0:128 of the odd col-group are never
                            # written (k-tile 1 is causally dead there)
                            nc.vector.memset(oaccB[64:128, 0:128], 0.0)
                    oaccA, oaccB = oaccs[(p, j)]
                    sl = i % 2
                    oacc = oaccB if sl else oaccA
                    nc.tensor.matmul(
                        out=oacc[64 * sl : 64 * sl + 64, off:512],
                        lhsT=xN[:, HD * i : HD * i + HD],
                        rhs=att[:, cur + off - off_sc : cur + 512 - off_sc],
                        start=(i <= 1),
                        stop=(i >= 4 * j + 2),
                    )
                    if i == 4 * j + 3:
                        # fold col-group halves: outs = A + B.  B is moved
                        # PSUM->SBUF by DMA (tensor_tensor cannot read two
                        # PSUM operands); sign(bv_v) is already in xN.
                        # read oaccA in the FIRST fold op: the next j's
                        # leading mm2 (start=True) reuses oaccA's PSUM slot
                        # (bufs=1 pool), so freeing A one DVE-op earlier
                        # shortens the PE stall at every j transition
                        bs = outp.tile([HD, 512], F32, name="bs", tag="bs")
                        nc.vector.tensor_scalar_mul(bs, oaccA[0:64, :], 1.0)
                        outs = outp.tile([HD, 512], F32, name="outs", tag="outs")
                        nc.vector.scalar_tensor_tensor(
                            out=outs,
                            in0=oaccB[64:128, :],
                            scalar=1.0,
                            in1=bs,
                            op0=mybir.AluOpType.mult,
                            op1=mybir.AluOpType.add,
                        )
                        nc.sync.dma_start(
                            out=out_d[p, :, 512 * j : 512 * j + 512], in_=outs
                        )
                        del oaccs[(p, j)]

            for p in range(PAIRS):
                load_pair(p)

            # mm2 emission is deferred and BATCHED two groups at a time: the
            # PE packs adjacent same-type matmuls into disjoint row/column
            # groups, so long runs of mm1s (or mm2s) pair ~perfectly while a
            # 3+3 interleave strands one unpaired matmul per triplet.
            pending = []
            all_groups = _groups()
            n_groups = len(all_groups)
            for gi, (chunk_list, gw, off0) in enumerate(all_groups):
                sc = psum_s.tile([128, SCW], F32)
                for (p, j, i, off, off_sc), cur in chunk_list:
                    xT, _, wxT = state[p]
                    bp = 64 * (i % 2)  # row-group base partition
                    nc.tensor.matmul(
                        out=sc[:, cur : cur + 512 - off_sc],
                        lhsT=wxT[bp : bp + 64, 128 * i : 128 * i + 128],
                        rhs=xT[bp : bp + 64, 512 * j + off_sc : 512 * j + 512],
                        start=True,
                        stop=True,
                    )
                if len(pending) >= 2 or (pending and gi >= n_groups - 3):
                    for chunks_a, att_a in pending:
                        emit_mm2(chunks_a, att_a)
                    pending = []
                att = attnp.tile([128, SCW], BF16)
                nc.scalar.activation(out=att[:, off0:gw], in_=sc[:, off0:gw], func=SIG)
                # causal staircase on the diagonal 128x128 blocks
                for (p, j, i, off, off_sc), cur in chunk_list:
                    if i >= 4 * j:
                        blk = slice(cur + off - off_sc, cur + off - off_sc + 128)
                        nc.vector.tensor_tensor(
                            out=att[:, blk],
                            in0=att[:, blk],
                            in1=stair,
                            op=mybir.AluOpType.mult,
                        )
                pending.append((chunk_list, att))
            for chunks_a, att_a in pending:
                emit_mm2(chunks_a, att_a)
    nc.compile()
    return nc


_CACHE: dict = {}


def _get_nc():
    if "nc" not in _CACHE:
        _CACHE["nc"] = build()
    return _CACHE["nc"]


def _make_in_maps(x, bv_q, bv_k, bv_v):
    x = np.asarray(x, dtype=np.float32)
    bv_q = np.asarray(bv_q, dtype=np.float32)
    bv_k = np.asarray(bv_k, dtype=np.float32)
    bv_v = np.asarray(bv_v, dtype=np.float32)
    w = 0.5 * np.sign(bv_q) * np.sign(bv_k)
    sv = np.sign(bv_v)

    pi = np.arange(128)
    msk = (pi[None, :] >= pi[:, None]).astype(ml_dtypes.bfloat16)  # stair01[p, n]

    in_maps = []
    for c in range(N_CORES):
        xT = np.empty((PAIRS, 128, T), ml_dtypes.bfloat16)
        wxT = np.empty((PAIRS, 128, T), ml_dtypes.bfloat16)
        xN = np.empty((PAIRS, 128, KT * HD), ml_dtypes.bfloat16)
        for p in range(PAIRS):
            g = PAIRS * c + p
            b, h = divmod(g, H)
            xs = x[b, :, HD * h : HD * h + HD]  # [T, HD]
            # swizzle with sv folded in: xN[pp, 64*k+d] = xs[128*k+pp, d]*sv[d]
            xsv = (xs * sv[h][None, :]).astype(ml_dtypes.bfloat16)
            xN[p] = xsv.reshape(KT, 128, HD).transpose(1, 0, 2).reshape(128, KT * HD)
            xsT = xs.T.astype(ml_dtypes.bfloat16)
            xT[p, 0:HD] = xsT
            xT[p, HD:128] = xsT
            # *(+-0.5) is an exact exponent shift in bf16
            wxT[p, 0:HD] = (xs.T * w[h][:, None]).astype(ml_dtypes.bfloat16)
            wxT[p, HD:128] = wxT[p, 0:HD]
        in_maps.append({"xT": xT, "wxT": wxT, "xN": xN, "msk": msk})
    return in_maps


def _assemble(results):
    out = np.empty((B, T, D), np.float32)
    for c in range(N_CORES):
        oT = results[c]["outT"]  # [PAIRS, HD, T]
        for p in range(PAIRS):
            g = PAIRS * c + p
            b, h = divmod(g, H)
            out[b, :, HD * h : HD * h + HD] = oT[p].T
    return out


def _run(x, bv_q, bv_k, bv_v, **spmd_kwargs):
    in_maps = _make_in_maps(x, bv_q, bv_k, bv_v)
    res = run_bass_kernel_spmd(
        _get_nc(), in_maps, core_ids=list(range(N_CORES)), **spmd_kwargs
    )
    return _assemble(res.results), res


def kernel(x, bv_q, bv_k, bv_v):
    out, _ = _run(x, bv_q, bv_k, bv_v)
    return out
